# revision 1
# baseline (speedup 1.0000x reference)
"""4-layer GCN (DglGCNNet) Trainium2 kernel, 8 NeuronCores.

Strategy (dst-partitioned graph; halo exchange == AllGather since the graph
is uniform random):
  - Host: bin-pack nodes into 8*98 blocks of <=128 dst nodes each, balancing
    per-block in-edge counts.  Core c owns 98 blocks (12544 padded node
    slots).  Edges are grouped by (dst block, src sub-table) and padded to
    128-edge tiles.
  - Device, per layer:
      A: h = X @ W per 128-node chunk on PE (X kept feat-major in SBUF,
         norm_src pre-folded into X rows), cast fp16, DMA to DRAM.
      B: AllGather h across the 8 cores -> fp16 table [100352, 128].
      C: per 2-block group: dma_gather edge messages from the table (one
         call per src sub-table of 32768 rows -- int16 index range); build
         one-hot indicator tiles from slot ids with a broadcast is_equal on
         DVE; segment-sum via indicator matmuls accumulating in PSUM;
         epilogue: *norm_dst + bias, tanh, *next-layer norm_src,
         PE-transpose back into the feat-major X buffer.
"""

import numpy as np

import concourse.bass as bass
import concourse.mybir as mybir
import concourse.tile as tile
from concourse import bacc

P = 128
D_IN = 128
D_HID = 128
D_OUT = 64
N_LAYERS = 4
G = 2  # dst blocks per gather-call group


class Cfg:
    def __init__(self, n_nodes, n_cores, blocks_per_core, subsz=32768):
        self.N = n_nodes
        self.NCORES = n_cores
        self.NBLK = blocks_per_core
        self.NP_CORE = blocks_per_core * P
        self.NPAD = n_cores * self.NP_CORE
        self.SUBSZ = subsz
        self.SUBS = list(range(0, self.NPAD, subsz))  # sub-table bases
        assert self.NPAD >= n_nodes
        assert self.NBLK % G == 0


FULL_CFG = Cfg(n_nodes=100000, n_cores=8, blocks_per_core=98)


# ---------------------------------------------------------------- host side


def degree_norms(edge_index, n):
    src = np.asarray(edge_index[0], dtype=np.int64)
    dst = np.asarray(edge_index[1], dtype=np.int64)
    out_deg = np.bincount(src, minlength=n).astype(np.float32)
    in_deg = np.bincount(dst, minlength=n).astype(np.float32)
    norm_src = np.where(out_deg > 0, 1.0 / np.sqrt(np.maximum(out_deg, 1.0)),
                        0.0).astype(np.float32)
    norm_dst = np.where(in_deg > 0, 1.0 / np.sqrt(np.maximum(in_deg, 1.0)),
                        0.0).astype(np.float32)
    return norm_src, norm_dst


def preprocess(edge_index, features, norms, cfg):
    """Partition the graph; build per-core device inputs.

    Returns (in_maps, kq, pos_of).
    """
    N, NCORES, NBLK, NP_CORE, NPAD = (
        cfg.N, cfg.NCORES, cfg.NBLK, cfg.NP_CORE, cfg.NPAD)
    norm_src, norm_dst = norms
    src = np.asarray(edge_index[0], dtype=np.int64)
    dst = np.asarray(edge_index[1], dtype=np.int64)
    in_deg = np.bincount(dst, minlength=N).astype(np.int64)

    # --- bin-pack nodes into NB blocks (<=P nodes each), balancing edges
    NB = NCORES * NBLK
    import heapq
    order = np.argsort(-in_deg, kind="stable")
    heap = [(0, b) for b in range(NB)]
    heapq.heapify(heap)
    counts = np.zeros(NB, np.int64)
    block_of = np.empty(N, np.int32)
    slot_of = np.empty(N, np.int32)
    for n in order:
        while True:
            w, b = heapq.heappop(heap)
            if counts[b] < P:
                break
        block_of[n] = b
        slot_of[n] = counts[b]
        counts[b] += 1
        heapq.heappush(heap, (w + int(in_deg[n]), b))

    block_w = np.bincount(block_of, weights=in_deg.astype(np.float64),
                          minlength=NB).astype(np.int64)

    # --- blocks -> cores (snake by weight to balance per-core edge totals)
    worder = np.argsort(-block_w, kind="stable")
    core_of_block = np.empty(NB, np.int32)
    idx_in_core = np.empty(NB, np.int32)
    fill = np.zeros(NCORES, np.int32)
    for i, b in enumerate(worder):
        rnd, j = divmod(i, NCORES)
        c = j if rnd % 2 == 0 else NCORES - 1 - j
        core_of_block[b] = c
        idx_in_core[b] = fill[c]
        fill[c] += 1

    pos_of = (core_of_block[block_of].astype(np.int64) * NP_CORE
              + idx_in_core[block_of].astype(np.int64) * P
              + slot_of.astype(np.int64))

    # --- per-(core, block, sub-table) edge counts -> caps
    NSUB = len(cfg.SUBS)
    pos_src = pos_of[src]
    q_of_edge = pos_src // cfg.SUBSZ
    e_blk = block_of[dst]
    e_core = core_of_block[e_blk]
    e_bic = idx_in_core[e_blk]

    cnt = np.zeros((NCORES, NBLK, NSUB), np.int64)
    np.add.at(cnt, (e_core, e_bic, q_of_edge), 1)
    kq = [int(-(-cnt[:, :, q].max() // P)) for q in range(NSUB)]
    kt = sum(kq)
    qoff_tiles = np.concatenate([[0], np.cumsum(kq)]).astype(np.int64)

    in_maps = []
    for c in range(NCORES):
        m = e_core == c
        bb = e_bic[m].astype(np.int64)
        qq = q_of_edge[m]
        ps = pos_src[m]
        sl = slot_of[dst[m]]
        # sort by (block, quadrant, src) for gather locality
        o = np.argsort((bb * NSUB + qq) * NPAD + ps, kind="stable")
        bb, qq, ps, sl = bb[o], qq[o], ps[o], sl[o]

        # per-(b, q) destination slot ranges within the padded edge stream
        seg = bb * NSUB + qq
        seg_cnt = np.bincount(seg, minlength=NBLK * NSUB).reshape(NBLK, NSUB)
        slots_q = np.array([k * P for k in kq], np.int64)
        seg_start = (np.arange(NBLK)[:, None] * (kt * P)
                     + np.concatenate([[0], np.cumsum(slots_q)])[:-1][None, :])
        starts_flat = seg_start.reshape(-1)
        cum = np.zeros(NBLK * NSUB, np.int64)
        cum[1:] = np.cumsum(seg_cnt.reshape(-1))[:-1]
        eslot = starts_flat[seg] + (np.arange(len(bb)) - cum[seg])

        # padded edge stream arrays (slot=255 kills padding in the indicator)
        tot = NBLK * kt * P
        idx16 = np.zeros(tot, np.int16)
        slotv = np.full(tot, 255.0, np.float16)
        idx16[eslot] = (ps - np.asarray(cfg.SUBS, np.int64)[qq]).astype(
            np.int16)
        slotv[eslot] = sl.astype(np.float16)

        # slot tensor [P, NBLK*KT]: tile t of block b <- edges [t*128, ...)
        slot_arr = np.ascontiguousarray(
            slotv.reshape(NBLK * kt, P).T)

        # gather index tensor, compact [16, COLS]; call (group, q) covers
        # G consecutive blocks' (b, q) segments concatenated
        ngrp = NBLK // G
        gw = G * kt * P // 16  # int16 cols per group
        gidxc = np.zeros((16, ngrp * gw), np.int16)
        stream = idx16.reshape(NBLK, kt * P)
        for g in range(ngrp):
            parts = []
            for q in range(NSUB):
                s0 = int(qoff_tiles[q]) * P
                s1 = int(qoff_tiles[q + 1]) * P
                for b01 in range(G):
                    parts.append(stream[g * G + b01, s0:s1])
            flat = np.concatenate(parts)
            j = np.arange(len(flat))
            a16 = np.zeros((16, len(flat) // 16), np.int16)
            a16[j % 16, j // 16] = flat
            gidxc[:, g * gw:(g + 1) * gw] = a16

        in_maps.append({"gidxc": gidxc, "slot": slot_arr})

    # --- node-order-dependent arrays
    xpad = np.zeros((NPAD, D_IN), np.float32)
    xpad[pos_of] = np.asarray(features, np.float32) * norm_src[:, None]
    nsrc_pad = np.zeros(NPAD, np.float32)
    nsrc_pad[pos_of] = norm_src
    ndst_pad = np.zeros(NPAD, np.float32)
    ndst_pad[pos_of] = norm_dst
    for c in range(NCORES):
        s = slice(c * NP_CORE, (c + 1) * NP_CORE)
        in_maps[c]["x0T"] = np.ascontiguousarray(xpad[s].T)
        in_maps[c]["nsrc"] = np.ascontiguousarray(
            nsrc_pad[s].reshape(NBLK, P).T)
        in_maps[c]["ndst"] = np.ascontiguousarray(
            ndst_pad[s].reshape(NBLK, P).T)

    return in_maps, kq, pos_of


def make_in_maps(inputs, cfg):
    norms = degree_norms(inputs["edge_index"], cfg.N)
    in_maps, kq, pos_of = preprocess(
        inputs["edge_index"], inputs["features"], norms, cfg)
    iota = np.tile(np.arange(P, dtype=np.float16), (P, 1))
    ident = np.eye(P, dtype=np.float32)
    for m in in_maps:
        m["iota"] = iota
        m["ident"] = ident
        for l in range(N_LAYERS):
            W = np.asarray(inputs[f"W{l}"], np.float32)
            b = np.asarray(inputs[f"b{l}"], np.float32)
            if W.shape[1] < D_IN:  # pad last layer to width 128
                W = np.pad(W, ((0, 0), (0, D_IN - W.shape[1])))
                b = np.pad(b, (0, D_IN - b.shape[0]))
            m[f"W{l}"] = W
            m[f"bb{l}"] = np.ascontiguousarray(
                np.broadcast_to(b, (P, D_IN)))
    return in_maps, kq, pos_of


def assemble_output(results, pos_of, cfg):
    full = np.concatenate([r["y"] for r in results], axis=0)
    return np.ascontiguousarray(full[pos_of])


# -------------------------------------------------------------- device side


def build_nc(cfg, kq):
    NCORES, NBLK, NP_CORE, NPAD = cfg.NCORES, cfg.NBLK, cfg.NP_CORE, cfg.NPAD
    NSUB = len(cfg.SUBS)
    assert NSUB == len(kq)
    kt = sum(kq)
    ngrp = NBLK // G
    gw = G * kt * P // 16
    T = NBLK * kt
    D = D_IN
    f32, f16, i16 = mybir.dt.float32, mybir.dt.float16, mybir.dt.int16
    qoff_tiles = np.concatenate([[0], np.cumsum(kq)]).astype(int)

    nc = bacc.Bacc("TRN2", target_bir_lowering=False, debug=False,
                   num_devices=NCORES)

    x0T_d = nc.dram_tensor("x0T", [D, NP_CORE], f32, kind="ExternalInput")
    gidxc_d = nc.dram_tensor("gidxc", [16, ngrp * gw], i16,
                             kind="ExternalInput")
    slot_d = nc.dram_tensor("slot", [P, T], f16, kind="ExternalInput")
    nsrc_d = nc.dram_tensor("nsrc", [P, NBLK], f32, kind="ExternalInput")
    ndst_d = nc.dram_tensor("ndst", [P, NBLK], f32, kind="ExternalInput")
    iota_d = nc.dram_tensor("iota", [P, P], f16, kind="ExternalInput")
    ident_d = nc.dram_tensor("ident", [P, P], f32, kind="ExternalInput")
    W_d = [nc.dram_tensor(f"W{l}", [D, D], f32, kind="ExternalInput")
           for l in range(N_LAYERS)]
    B_d = [nc.dram_tensor(f"bb{l}", [P, D], f32, kind="ExternalInput")
           for l in range(N_LAYERS)]
    y_d = nc.dram_tensor("y", [NP_CORE, D_OUT], f32, kind="ExternalOutput")

    hloc = [nc.dram_tensor(f"hloc{i}", [NP_CORE, D], f16) for i in range(2)]
    hful = [nc.dram_tensor(f"hful{i}", [NPAD, D], f16, addr_space="Shared")
            for i in range(2)]

    # persistent SBUF
    xT = [nc.alloc_sbuf_tensor(f"xT{i}", [D, NP_CORE], f32).ap()
          for i in range(2)]
    slot_s = nc.alloc_sbuf_tensor("slot_s", [P, T], f16).ap()
    nsrc_s = nc.alloc_sbuf_tensor("nsrc_s", [P, NBLK], f32).ap()
    ndst_s = nc.alloc_sbuf_tensor("ndst_s", [P, NBLK], f32).ap()
    iota_s = nc.alloc_sbuf_tensor("iota_s", [P, P], f16).ap()
    ident_s = nc.alloc_sbuf_tensor("ident_s", [P, P], f32).ap()
    W_s = [nc.alloc_sbuf_tensor(f"W_s{l}", [D, D], f32).ap()
           for l in range(N_LAYERS)]
    B_s = [nc.alloc_sbuf_tensor(f"B_s{l}", [P, D], f32).ap()
           for l in range(N_LAYERS)]

    rg = [list(range(NCORES))]

    def bcast16(dram, col0, w):
        """AP reading [16, w] at col0 replicated 8x -> [128, w]."""
        a = dram[:, col0:col0 + w]
        return bass.AP(a.tensor, a.offset, [[0, 8]] + list(a.ap))

    with tile.TileContext(nc) as tc:
        with (
            tc.tile_pool(name="gip", bufs=3) as gip,
            tc.tile_pool(name="msgp", bufs=2) as msgp,
            tc.tile_pool(name="indp", bufs=3) as indp,
            tc.tile_pool(name="hap", bufs=4) as hap,
            tc.tile_pool(name="epp", bufs=4) as epp,
            tc.tile_pool(name="psA", bufs=2, space="PSUM") as psA,
            tc.tile_pool(name="psC", bufs=2, space="PSUM") as psC,
            tc.tile_pool(name="psT", bufs=2, space="PSUM") as psT,
        ):
            # ---- load constants
            nc.sync.dma_start(out=xT[0], in_=x0T_d[:, :])
            nc.sync.dma_start(out=slot_s, in_=slot_d[:, :])
            nc.sync.dma_start(out=nsrc_s, in_=nsrc_d[:, :])
            nc.sync.dma_start(out=ndst_s, in_=ndst_d[:, :])
            nc.sync.dma_start(out=iota_s, in_=iota_d[:, :])
            nc.sync.dma_start(out=ident_s, in_=ident_d[:, :])
            for l in range(N_LAYERS):
                nc.sync.dma_start(out=W_s[l], in_=W_d[l][:, :])
                nc.sync.dma_start(out=B_s[l], in_=B_d[l][:, :])

            for l in range(N_LAYERS):
                last = l == N_LAYERS - 1
                xcur = xT[l % 2]
                xnext = xT[(l + 1) % 2]
                hl = hloc[l % 2]
                hf = hful[l % 2]

                # ---- A: h = X @ W (node-major chunks), cast fp16, to DRAM
                for b in range(NBLK):
                    ph = psA.tile([P, D], f32, tag="psA")
                    nc.tensor.matmul(ph[:], lhsT=xcur[:, b * P:(b + 1) * P],
                                     rhs=W_s[l][:, :], start=True, stop=True)
                    hsb = hap.tile([P, D], f16, tag="h")
                    nc.vector.tensor_copy(out=hsb[:], in_=ph[:])
                    nc.sync.dma_start(out=hl[b * P:(b + 1) * P, :],
                                      in_=hsb[:])

                # ---- B: AllGather
                nc.gpsimd.collective_compute(
                    "AllGather", mybir.AluOpType.bypass, replica_groups=rg,
                    ins=[hl[:, :]], outs=[hf[:, :]])

                # ---- C: gather + segment-sum + epilogue per 2-block group
                for g in range(ngrp):
                    gi = gip.tile([P, gw], i16, tag="gi")
                    nc.sync.dma_start(out=gi[:],
                                      in_=bcast16(gidxc_d, g * gw, gw))
                    msg = msgp.tile([P, G * kt * D], f16, tag="msg")
                    coff = 0  # int16 col offset into gi
                    moff = 0  # tile offset into msg
                    for q in range(NSUB):
                        if kq[q] == 0:
                            continue
                        nidx = G * kq[q] * P
                        sub = hf[cfg.SUBS[q]:
                                 min(cfg.SUBS[q] + cfg.SUBSZ, NPAD), :]
                        nc.gpsimd.dma_gather(
                            out_ap=msg[:, moff * D:(moff + G * kq[q]) * D]
                            .rearrange("p (t e) -> p t e", e=D),
                            in_ap=sub,
                            idxs_ap=gi[:, coff:coff + nidx // 16],
                            num_idxs=nidx,
                            num_idxs_reg=nidx,
                            elem_size=D,
                            single_packet=False)
                        coff += nidx // 16
                        moff += G * kq[q]

                    for b01 in range(G):
                        b = g * G + b01
                        ind = indp.tile([P, kt * P], f16, tag="ind")
                        ind_ap = ind[:]
                        ind3 = bass.AP(ind_ap.tensor, ind_ap.offset,
                                       [[kt * P, P], [P, kt], [1, P]])
                        slot3 = slot_s[:, b * kt:(b + 1) * kt].to_broadcast(
                            [P, kt, P])
                        iota3 = bass.AP(iota_s.tensor, iota_s.offset,
                                        [[P, P], [0, kt], [1, P]])
                        nc.vector.tensor_tensor(
                            out=ind3, in0=slot3, in1=iota3,
                            op=mybir.AluOpType.is_equal)

                        pagg = psC.tile([P, D], f32, tag="psC")
                        for t in range(kt):
                            q = int(np.searchsorted(qoff_tiles, t,
                                                    side="right")) - 1
                            j = t - int(qoff_tiles[q])
                            mcol = (G * int(qoff_tiles[q])
                                    + b01 * kq[q] + j)
                            nc.tensor.matmul(
                                pagg[:],
                                lhsT=ind[:, t * P:(t + 1) * P],
                                rhs=msg[:, mcol * D:(mcol + 1) * D],
                                start=(t == 0), stop=(t == kt - 1))

                        t1 = epp.tile([P, D], f32, tag="t1")
                        nc.vector.tensor_scalar(
                            out=t1[:], in0=pagg[:],
                            scalar1=ndst_s[:, b:b + 1], scalar2=None,
                            op0=mybir.AluOpType.mult)
                        nc.vector.tensor_add(out=t1[:], in0=t1[:],
                                             in1=B_s[l][:, :])
                        if last:
                            nc.sync.dma_start(
                                out=y_d[b * P:(b + 1) * P, :],
                                in_=t1[:, :D_OUT])
                        else:
                            t2 = epp.tile([P, D], f32, tag="t2")
                            nc.scalar.activation(
                                out=t2[:], in_=t1[:],
                                func=mybir.ActivationFunctionType.Tanh)
                            nc.vector.tensor_scalar(
                                out=t2[:], in0=t2[:],
                                scalar1=nsrc_s[:, b:b + 1], scalar2=None,
                                op0=mybir.AluOpType.mult)
                            pt = psT.tile([P, P], f32, tag="psT")
                            nc.tensor.transpose(pt[:], t2[:], ident_s)
                            nc.vector.tensor_copy(
                                out=xnext[:, b * P:(b + 1) * P], in_=pt[:])

    nc.compile()
    return nc


_CACHE = {}
LAST_EXEC_NS = None


def kernel(**inputs):
    global LAST_EXEC_NS
    from concourse.bass_utils import run_bass_kernel_spmd

    cfg = FULL_CFG
    in_maps, kq, pos_of = make_in_maps(inputs, cfg)
    key = ("full", tuple(kq))
    if key not in _CACHE:
        _CACHE[key] = build_nc(cfg, kq)
    nc = _CACHE[key]
    res = run_bass_kernel_spmd(nc, in_maps, list(range(cfg.NCORES)))
    LAST_EXEC_NS = res.exec_time_ns
    out = assemble_output(res.results, pos_of, cfg)
    return out.astype(np.float32)



# revision 3
# speedup vs baseline: 3.8999x; 3.8999x over previous
"""4-layer GCN (DglGCNNet) Trainium2 kernel, 8 NeuronCores. v3.

Instruction-count-minimal design (the axon/PJRT execution path costs
~30-70us per instruction, so the kernel is instruction-bound, not
FLOP-bound):

  Host: sort nodes by in-degree desc, deal round-robin to 8 cores; each
  core's 12544 node slots form 98 degree bands of 128 nodes with per-band
  edge capacity K_b (max in-degree in the band, globally uniform).  The
  edge buffer (ebuf) has one unique slot per (dst, j<K_b): slot =
  BB_b + j*128 + p.  Edges are streamed sorted by (ebuf segment of 32768,
  src subtable of 25088, src), padded per (seg, subtable) to globally
  uniform lengths; pad entries gather a guaranteed-zero h row and
  scatter to the seg's unused slots, so every ebuf slot is written
  exactly once per layer (no zeroing, no races).

  Device, per layer:
    A: h = X @ W per 128-node block on PE (X feat-major fp16 in SBUF),
       4 blocks share one PSUM bank -> 1 copy + 1 DMA per 512 nodes.
    B: AllGather h -> fp16 table [100352, 128].
    C: per <=4096-edge chunk: one idx DMA, one dma_gather (messages from
       the table), one dma_scatter_add into ebuf (all slots unique ->
       no RMW collisions).  Then per equal-K band group: 1 readback DMA,
       1 strided tensor_reduce over j (segment sum!), 1 DMA to agg.
       One dma_start_transpose pulls agg back feat-major, and the whole
       epilogue (*norm_dst, tanh+bias, *norm_src) is 3 bulk ops into the
       next layer's X buffer.
"""

import numpy as np

import concourse.bass as bass
import concourse.mybir as mybir
import concourse.tile as tile
from concourse import bacc

P = 128
D_IN = 128
D_HID = 128
D_OUT = 64
N_LAYERS = 4
SEG = 32768          # ebuf segment (scatter int16 index range)
import os as _os
CHUNK = int(_os.environ.get("V3_CHUNK", "4096"))  # max idx per SWDGE call
RB_SLOTS = int(_os.environ.get("V3_RB", "8192"))  # max slots per reduce group
V3_SP = _os.environ.get("V3_SP", "0") == "1"
V3_GBUFS = int(_os.environ.get("V3_GBUFS", "3"))


class Cfg:
    def __init__(self, n_nodes, n_cores, blocks_per_core, subsz=25088):
        self.N = n_nodes
        self.NCORES = n_cores
        self.NBLK = blocks_per_core
        self.NP_CORE = blocks_per_core * P
        self.NPAD = n_cores * self.NP_CORE
        self.SUBSZ = subsz
        self.SUBS = list(range(0, self.NPAD, subsz))
        assert self.NPAD % subsz == 0 and subsz <= 32768
        assert self.NPAD >= n_nodes


FULL_CFG = Cfg(n_nodes=100000, n_cores=8, blocks_per_core=98)


# ---------------------------------------------------------------- host side


def degree_norms(edge_index, n):
    src = np.asarray(edge_index[0], dtype=np.int64)
    dst = np.asarray(edge_index[1], dtype=np.int64)
    out_deg = np.bincount(src, minlength=n).astype(np.float32)
    in_deg = np.bincount(dst, minlength=n).astype(np.float32)
    norm_src = np.where(out_deg > 0, 1.0 / np.sqrt(np.maximum(out_deg, 1.0)),
                        0.0).astype(np.float32)
    norm_dst = np.where(in_deg > 0, 1.0 / np.sqrt(np.maximum(in_deg, 1.0)),
                        0.0).astype(np.float32)
    return norm_src, norm_dst


def preprocess(edge_index, features, norms, cfg):
    """Returns (in_maps, plan, pos_of).

    plan: dict with K (per-band capacities), BB (band slot bases), S
    (ebuf slots), chunks [(q, seg, off, n)], rgroups [(b0, nb, K)].
    """
    N, NCORES, NP_CORE, NPAD = cfg.N, cfg.NCORES, cfg.NP_CORE, cfg.NPAD
    NBLK, SUBSZ = cfg.NBLK, cfg.SUBSZ
    NSUB = NPAD // SUBSZ
    norm_src, norm_dst = norms
    src = np.asarray(edge_index[0], dtype=np.int64)
    dst = np.asarray(edge_index[1], dtype=np.int64)
    deg = np.bincount(dst, minlength=N).astype(np.int64)

    # --- nodes: sort by in-degree desc, deal round-robin to cores
    order = np.argsort(-deg, kind="stable")           # rank -> node
    pos_of = np.empty(N, np.int64)
    r = np.arange(N)
    pos_of[order] = (r % NCORES) * NP_CORE + r // NCORES
    deg_sorted = deg[order]

    # --- band capacities K_b (uniform across cores), ebuf multiple of SEG
    K = np.zeros(NBLK, np.int64)
    for b in range(NBLK):
        K[b] = max(int(deg_sorted[b * P * NCORES]), 1)
    per_seg_k = SEG // P                               # 256
    SK_target = -(-int(K.sum()) // per_seg_k) * per_seg_k
    bump = SK_target - int(K.sum())
    K += bump // NBLK
    rem = bump % NBLK
    if rem > 0:
        K[NBLK - rem:] += 1
    S = int(K.sum()) * P
    NSEGS = S // SEG
    BB = np.concatenate([[0], np.cumsum(K)]) * P       # band slot bases

    # --- per-edge core/slot
    pos_dst = pos_of[dst]
    pos_src = pos_of[src]
    e_core = pos_dst // NP_CORE
    loc = pos_dst % NP_CORE
    e_b = loc // P
    e_p = loc % P
    # j = per-dst edge counter: sort edges by pos_dst
    od = np.argsort(pos_dst, kind="stable")
    pd_sorted = pos_dst[od]
    starts = np.searchsorted(pd_sorted, pos_dst)       # first occurrence idx
    jj = np.empty(len(src), np.int64)
    jj[od] = np.arange(len(src)) - starts[od]
    assert (jj < K[e_b]).all()
    slot = BB[e_b] + jj * P + e_p
    e_seg = slot // SEG
    e_q = pos_src // SUBSZ

    # --- per-(seg, q) padded run lengths (uniform across cores)
    def segsize(s):
        return min(SEG, S - s * SEG)

    cnt = np.zeros((NCORES, NSEGS, NSUB), np.int64)
    np.add.at(cnt, (e_core, e_seg, e_q), 1)
    L = -(-cnt.max(axis=0) // P) * P                   # [NSEGS, NSUB]
    for s in range(NSEGS):
        slack = segsize(s) - int(L[s].sum())
        assert slack >= 0, f"seg {s}: padded runs exceed segment"
        L[s, 0] += slack
    assert (L.sum(axis=1) == [segsize(s) for s in range(NSEGS)]).all()

    # --- zero h rows per subtable (positions of padding node slots)
    pad_pos = np.setdiff1d(
        np.arange(NPAD, dtype=np.int64), pos_of, assume_unique=False)
    zrow = np.full(NSUB, -1, np.int64)
    for q in range(NSUB):
        cand = pad_pos[(pad_pos >= q * SUBSZ) & (pad_pos < (q + 1) * SUBSZ)]
        assert len(cand) > 0, f"no zero row in subtable {q}"
        zrow[q] = cand[0]

    # --- chunk schedule (uniform): (q, seg, stream offset, n)
    chunks = []
    off = 0
    run_off = np.zeros((NSEGS, NSUB), np.int64)
    for s in range(NSEGS):
        for q in range(NSUB):
            run_off[s, q] = off
            n = int(L[s, q])
            o = 0
            while o < n:
                c = min(CHUNK, n - o)
                chunks.append((q, s, off + o, c))
                o += c
            off += n
    assert off == S

    # --- per-core gather/scatter index streams
    in_maps = []
    for c in range(NCORES):
        m = e_core == c
        ps_c = pos_src[m]
        sl_c = slot[m]
        seg_c = e_seg[m]
        q_c = e_q[m]
        o = np.argsort((seg_c * NSUB + q_c) * (NPAD + 1) + ps_c,
                       kind="stable")
        ps_c, sl_c, seg_c, q_c = ps_c[o], sl_c[o], seg_c[o], q_c[o]

        gidx = np.empty(S, np.int64)
        sidx = np.empty(S, np.int64)
        # unused slots per seg, for pad entries
        used = np.zeros(S, bool)
        used[sl_c] = True
        ei = 0
        for s in range(NSEGS):
            free_slots = np.flatnonzero(~used[s * SEG:(s + 1) * SEG]) + s * SEG
            fi = 0
            for q in range(NSUB):
                n_real = int(cnt[c, s, q])
                n_padded = int(L[s, q])
                w = run_off[s, q]
                seg_edges = slice(ei, ei + n_real)
                assert (seg_c[seg_edges] == s).all() and \
                       (q_c[seg_edges] == q).all()
                gidx[w:w + n_real] = ps_c[seg_edges] - q * SUBSZ
                sidx[w:w + n_real] = sl_c[seg_edges] - s * SEG
                npad = n_padded - n_real
                gidx[w + n_real:w + n_padded] = zrow[q] - q * SUBSZ
                sidx[w + n_real:w + n_padded] = \
                    free_slots[fi:fi + npad] - s * SEG
                fi += npad
                ei += n_real
            assert fi == len(free_slots)
        assert ei == len(ps_c)
        assert (gidx >= 0).all() and (gidx < SUBSZ).all()
        assert (sidx >= 0).all() and (sidx < SEG).all()

        # interleave per chunk: [gather cols | scatter cols]
        idxw = np.zeros((16, 2 * S // 16), np.int16)
        for (q, s, coff, n) in chunks:
            g = gidx[coff:coff + n]
            t = sidx[coff:coff + n]
            j = np.arange(n)
            base = 2 * coff // 16
            a = np.zeros((16, n // 16), np.int16)
            a[j % 16, j // 16] = g.astype(np.int16)
            idxw[:, base:base + n // 16] = a
            a = np.zeros((16, n // 16), np.int16)
            a[j % 16, j // 16] = t.astype(np.int16)
            idxw[:, base + n // 16:base + 2 * (n // 16)] = a
        in_maps.append({"idx": idxw})

    # --- reduce groups: consecutive equal-K bands, capped slots
    rgroups = []
    b = 0
    while b < NBLK:
        k = int(K[b])
        nb = 1
        while (b + nb < NBLK and int(K[b + nb]) == k
               and (nb + 1) * k * P <= RB_SLOTS):
            nb += 1
        rgroups.append((b, nb, k))
        b += nb

    # --- node-order-dependent arrays
    xpad = np.zeros((NPAD, D_IN), np.float32)
    xpad[pos_of] = np.asarray(features, np.float32) * norm_src[:, None]
    nsrc_pad = np.zeros(NPAD, np.float32)
    nsrc_pad[pos_of] = norm_src
    ndst_pad = np.zeros(NPAD, np.float32)
    ndst_pad[pos_of] = norm_dst
    absmax = np.abs(xpad).max(axis=1)
    xscale = np.where(absmax > 0, absmax / 127.0, 1.0).astype(np.float32)
    x8 = np.clip(np.rint(xpad / xscale[:, None]), -127, 127).astype(np.int8)
    for c in range(NCORES):
        s = slice(c * NP_CORE, (c + 1) * NP_CORE)
        in_maps[c]["x8T"] = np.ascontiguousarray(x8[s].T)
        in_maps[c]["xscaleT"] = np.ascontiguousarray(
            xscale[s].astype(np.float16)[None, :])
        in_maps[c]["nsrcT"] = np.ascontiguousarray(
            nsrc_pad[s].astype(np.float16)[None, :])
        in_maps[c]["ndstT"] = np.ascontiguousarray(
            ndst_pad[s].astype(np.float16)[None, :])

    plan = {"K": K, "BB": BB, "S": S, "NSEGS": NSEGS,
            "chunks": chunks, "rgroups": rgroups}
    return in_maps, plan, pos_of


def make_in_maps(inputs, cfg):
    norms = degree_norms(inputs["edge_index"], cfg.N)
    in_maps, plan, pos_of = preprocess(
        inputs["edge_index"], inputs["features"], norms, cfg)
    for m in in_maps:
        for l in range(N_LAYERS):
            W = np.asarray(inputs[f"W{l}"], np.float32)
            b = np.asarray(inputs[f"b{l}"], np.float32)
            if W.shape[1] < D_IN:  # pad last layer to width 128
                W = np.pad(W, ((0, 0), (0, D_IN - W.shape[1])))
                b = np.pad(b, (0, D_IN - b.shape[0]))
            m[f"W{l}"] = W.astype(np.float16)
            m[f"bc{l}"] = np.ascontiguousarray(b[:, None])  # [128, 1] f32
    return in_maps, plan, pos_of


def assemble_output(results, pos_of, cfg):
    # y is [64, NP_CORE] fp16 feat-major per core
    per_core = [r["y"].T for r in results]
    full = np.concatenate(per_core, axis=0).astype(np.float32)
    return np.ascontiguousarray(full[pos_of])


# -------------------------------------------------------------- device side


def build_nc(cfg, plan, debug=False):
    NCORES, NBLK, NP_CORE, NPAD = cfg.NCORES, cfg.NBLK, cfg.NP_CORE, cfg.NPAD
    SUBSZ = cfg.SUBSZ
    K, BB, S = plan["K"], plan["BB"], plan["S"]
    chunks, rgroups = plan["chunks"], plan["rgroups"]
    D = D_IN
    f32, f16, i16 = mybir.dt.float32, mybir.dt.float16, mybir.dt.int16

    nc = bacc.Bacc("TRN2", target_bir_lowering=False, debug=False,
                   num_devices=NCORES)

    i8 = mybir.dt.int8
    x8T_d = nc.dram_tensor("x8T", [D, NP_CORE], i8, kind="ExternalInput")
    xscaleT_d = nc.dram_tensor("xscaleT", [1, NP_CORE], f16,
                               kind="ExternalInput")
    idx_d = nc.dram_tensor("idx", [16, 2 * S // 16], i16,
                           kind="ExternalInput")
    nsrcT_d = nc.dram_tensor("nsrcT", [1, NP_CORE], f16, kind="ExternalInput")
    ndstT_d = nc.dram_tensor("ndstT", [1, NP_CORE], f16, kind="ExternalInput")
    W_d = [nc.dram_tensor(f"W{l}", [D, D], f16, kind="ExternalInput")
           for l in range(N_LAYERS)]
    B_d = [nc.dram_tensor(f"bc{l}", [P, 1], f32, kind="ExternalInput")
           for l in range(N_LAYERS)]
    y_d = nc.dram_tensor("y", [D_OUT, NP_CORE], f16, kind="ExternalOutput")
    if debug:
        dbga = [nc.dram_tensor(f"dbga{l}", [P, NP_CORE], f16,
                               kind="ExternalOutput") for l in range(N_LAYERS)]
        dbgx = [nc.dram_tensor(f"dbgx{l}", [P, NP_CORE], f16,
                               kind="ExternalOutput") for l in range(N_LAYERS)]

    hloc = [nc.dram_tensor(f"hloc{i}", [NP_CORE, D], f16) for i in range(2)]
    hful = [nc.dram_tensor(f"hful{i}", [NPAD, D], f16, addr_space="Shared")
            for i in range(2)]
    ebuf_d2 = [nc.dram_tensor(f"ebuf{i}", [S, D], f16) for i in range(2)]
    agg_d2 = [nc.dram_tensor(f"agg{i}", [NP_CORE, D], f16)
              for i in range(2)]

    # persistent SBUF
    xT = [nc.alloc_sbuf_tensor(f"xT{i}", [D, NP_CORE], f16).ap()
          for i in range(2)]
    ndst_rep = nc.alloc_sbuf_tensor("ndst_rep", [P, NP_CORE], f16).ap()
    nsrc_rep = nc.alloc_sbuf_tensor("nsrc_rep", [P, NP_CORE], f16).ap()
    aggT = nc.alloc_sbuf_tensor("aggT", [P, NP_CORE], f16).ap()
    zsb = nc.alloc_sbuf_tensor("zsb", [P, P], f16).ap()
    W_s = [nc.alloc_sbuf_tensor(f"W_s{l}", [D, D], f16).ap()
           for l in range(N_LAYERS)]
    B_s = [nc.alloc_sbuf_tensor(f"B_s{l}", [P, 1], f32).ap()
           for l in range(N_LAYERS)]

    rg = [list(range(NCORES))]

    def bcast16(dram, col0, w):
        a = dram[:, col0:col0 + w]
        return bass.AP(a.tensor, a.offset, [[0, 8]] + list(a.ap))

    def bcast_row(dram, n):
        a = dram[:, :]
        return bass.AP(a.tensor, a.offset, [[0, P], [1, n]])

    with tile.TileContext(nc) as tc:
        with (
            tc.tile_pool(name="hap", bufs=2) as hap,
            tc.tile_pool(name="gip", bufs=2) as gip,
            tc.tile_pool(name="msgp", bufs=2) as msgp,
            tc.tile_pool(name="rbp", bufs=2) as rbp,
            tc.tile_pool(name="rop", bufs=2) as rop,
            tc.tile_pool(name="x8p", bufs=1) as x8p,
            tc.tile_pool(name="psA", bufs=2, space="PSUM") as psA,
        ):
            # ---- load constants
            nc.vector.memset(zsb, 0.0)
            nc.sync.dma_start(out=aggT, in_=bcast_row(xscaleT_d, NP_CORE))
            x8s = x8p.tile([D, NP_CORE], i8, tag="x8")
            nc.sync.dma_start(out=x8s[:], in_=x8T_d[:, :])
            nc.vector.tensor_copy(out=xT[0], in_=x8s[:])
            nc.vector.tensor_tensor(out=xT[0], in0=xT[0], in1=aggT,
                                    op=mybir.AluOpType.mult)
            nc.sync.dma_start(out=ndst_rep, in_=bcast_row(ndstT_d, NP_CORE))
            nc.sync.dma_start(out=nsrc_rep, in_=bcast_row(nsrcT_d, NP_CORE))
            for l in range(N_LAYERS):
                nc.sync.dma_start(out=W_s[l], in_=W_d[l][:, :])
                nc.sync.dma_start(out=B_s[l], in_=B_d[l][:, :])

            for l in range(N_LAYERS):
                last = l == N_LAYERS - 1
                xcur = xT[l % 2]
                xnext = xT[(l + 1) % 2]
                hl = hloc[l % 2]
                hf = hful[l % 2]
                ebuf_d = ebuf_d2[l % 2]
                agg_d = agg_d2[l % 2]

                # ---- A: h = X @ W, 4 blocks per PSUM bank
                for g0 in range(0, NBLK, 4):
                    gn = min(4, NBLK - g0)
                    ph = psA.tile([P, gn * D], f32, tag="psA")
                    for b01 in range(gn):
                        nc.tensor.matmul(
                            ph[:, b01 * D:(b01 + 1) * D],
                            lhsT=xcur[:, (g0 + b01) * P:(g0 + b01 + 1) * P],
                            rhs=W_s[l][:, :], start=True, stop=True)
                    hsb = hap.tile([P, gn * D], f16, tag="h")
                    nc.scalar.copy(out=hsb[:], in_=ph[:])
                    nc.sync.dma_start(
                        out=hl[g0 * P:(g0 + gn) * P, :]
                        .rearrange("(t p) e -> p t e", p=P),
                        in_=hsb[:].rearrange("p (t e) -> p t e", e=D))

                # ---- B: AllGather
                nc.gpsimd.collective_compute(
                    "AllGather", mybir.AluOpType.bypass, replica_groups=rg,
                    ins=[hl[:, :]], outs=[hf[:, :]])

                # ---- C1: zero ebuf, then gather + scatter per chunk
                zstep = -(-S // (4 * P)) * P
                for z0 in range(0, S, zstep):
                    zn = min(zstep, S - z0)
                    nc.sync.dma_start(
                        out=ebuf_d[z0:z0 + zn, :]
                        .rearrange("(p t) e -> p t e", p=P),
                        in_=bass.AP(zsb.tensor, zsb.offset,
                                    [list(zsb.ap[0]), [0, zn // P],
                                     [1, P]]))
                for (q, s, off, n) in chunks:
                    gi = gip.tile([P, 2 * n // 16], i16, tag="gi")
                    nc.sync.dma_start(
                        out=gi[:], in_=bcast16(idx_d, 2 * off // 16,
                                               2 * n // 16))
                    msg = msgp.tile([P, n], f16, tag="msg")
                    nc.gpsimd.dma_gather(
                        out_ap=msg[:].rearrange("p (t e) -> p t e", e=D),
                        in_ap=hf[q * SUBSZ:(q + 1) * SUBSZ, :],
                        idxs_ap=gi[:, 0:n // 16],
                        num_idxs=n, num_idxs_reg=n,
                        elem_size=D, single_packet=V3_SP)
                    nc.gpsimd.dma_scatter_add(
                        out_ap=ebuf_d[s * SEG:min((s + 1) * SEG, S), :],
                        in_ap=msg[:].rearrange("p (t e) -> p t e", e=D),
                        idxs_ap=gi[:, n // 16:2 * (n // 16)],
                        num_idxs=n, num_idxs_reg=n,
                        elem_size=D, single_packet=V3_SP)

                # ---- C2: per band group: readback + segment reduce + agg
                for (b0, nb, k) in rgroups:
                    r0 = int(BB[b0])
                    nrows = nb * k * P
                    rb = rbp.tile([P, nrows], f16, tag="rb")
                    a = ebuf_d[r0:r0 + nrows, :]
                    nc.sync.dma_start(
                        out=rb[:].rearrange("p (nb j e) -> p nb j e",
                                            nb=nb, j=k),
                        in_=bass.AP(a.tensor, a.offset,
                                    [[D, P], [k * P * D, nb],
                                     [P * D, k], [1, D]]))
                    ro = rop.tile([P, nb * D], f16, tag="ro")
                    rb_ap = rb[:]
                    in4 = bass.AP(rb_ap.tensor, rb_ap.offset,
                                  [[nrows, P], [k * D, nb],
                                   [1, D], [D, k]])
                    ro_ap = ro[:]
                    out3 = bass.AP(ro_ap.tensor, ro_ap.offset,
                                   [[nb * D, P], [D, nb], [1, D]])
                    with nc.allow_low_precision("segment sum of <=K fp16 "
                                                "h rows; |h|~O(1)"):
                        nc.vector.tensor_reduce(
                            out=out3, in_=in4,
                            axis=mybir.AxisListType.X,
                            op=mybir.AluOpType.add)
                    nc.sync.dma_start(
                        out=agg_d[b0 * P:(b0 + nb) * P, :]
                        .rearrange("(t p) e -> p t e", p=P),
                        in_=ro[:].rearrange("p (t e) -> p t e", e=D))

                # ---- C3: transpose readback + bulk epilogue
                nc.sync.dma_start_transpose(out=aggT, in_=agg_d[:, :])
                if debug:
                    nc.sync.dma_start(out=dbga[l][:, :], in_=aggT)
                if last:
                    nc.vector.tensor_tensor(
                        out=xnext[0:D_OUT, :], in0=aggT[0:D_OUT, :],
                        in1=ndst_rep[0:D_OUT, :], op=mybir.AluOpType.mult)
                    nc.vector.tensor_scalar(
                        out=xnext[0:D_OUT, :], in0=xnext[0:D_OUT, :],
                        scalar1=B_s[l][0:D_OUT, :], scalar2=None,
                        op0=mybir.AluOpType.add)
                    nc.sync.dma_start(out=y_d[:, :], in_=xnext[0:D_OUT, :])
                else:
                    nc.vector.tensor_tensor(
                        out=xnext, in0=aggT, in1=ndst_rep,
                        op=mybir.AluOpType.mult)
                    nc.scalar.activation(
                        out=xnext, in_=xnext,
                        func=mybir.ActivationFunctionType.Tanh,
                        bias=B_s[l][:, :])
                    nc.vector.tensor_tensor(
                        out=xnext, in0=xnext, in1=nsrc_rep,
                        op=mybir.AluOpType.mult)
                if debug:
                    nc.sync.dma_start(out=dbgx[l][:, :],
                                      in_=xnext if not last else xT[0])

    nc.compile()
    return nc


_CACHE = {}
LAST_EXEC_NS = None


def kernel(**inputs):
    global LAST_EXEC_NS
    from concourse.bass_utils import run_bass_kernel_spmd

    cfg = FULL_CFG
    in_maps, plan, pos_of = make_in_maps(inputs, cfg)
    key = ("v3", plan["S"], tuple(plan["K"].tolist()))
    if key not in _CACHE:
        _CACHE[key] = build_nc(cfg, plan)
    nc = _CACHE[key]
    res = run_bass_kernel_spmd(nc, in_maps, list(range(cfg.NCORES)))
    LAST_EXEC_NS = res.exec_time_ns
    out = assemble_output(res.results, pos_of, cfg)
    return out.astype(np.float32)
